# revision 30
# baseline (speedup 1.0000x reference)
"""CrossNetMix (moe_routing) Trainium2 Bass kernel — transfer-optimized.

Math (per layer i, softmax gates g sum to 1 over E):
    x_{l+1} = x_l + x0 * (sum_e g_e * U_e @ tanh(C_e @ tanh(V_e^T x_l)) + bias_i)

The residual chain collapses to x_L = x0 * u with
u = 1 + sum_i (gated_moe_i + bias_i), so the kernel carries u and
materializes y_i = y0 * u_i (transposed space y = x^T) as matmul input.

The axon host<->device tunnel is the whole game: ~50 MB/s per direction
for large transfers, degrading to ~46 MB/s AGGREGATE when both directions
run concurrently — so overlapping upload with download does not reduce
byte-time, and a single full-batch call beats sliced pipelining (measured:
1 slice 0.86s, 2 slices 0.93s, 4 slices 1.01s, 8 slices 1.40s).  On-device
compute is ~0.2 ms.  Transfer plan (~32 MB per call vs ~250 MB naive):
  - x uploads as int8 with a per-row scale (16 MB + 32 KB): dequantized to
    fp16 on device, then PE-transposed into column-major y0.
  - the device returns u (not y = x0*u) quantized int8 with per-row
    scales computed on device via abs-max reduce; the host computes
    x0_f32 * (uq * su), so the final product uses EXACT x0 —
    x-quantization only enters through the layer nonlinearities
    (attenuated), keeping l2 error ~4.8e-3 vs the 2e-2 gate.
  - all weights pack into one fp16 blob [128, 13112], sharded 8-ways
    (0.42 MB uploaded per core) and reassembled on device with a
    NeuronLink AllGather; constants (identity/select/ones) ship inside
    the NEFF via inline_tensor; the blob upload is skipped entirely when
    the weights are unchanged from the previous call (device-resident).
  - donated output buffers are created on device (no zeros upload); host
    quantize/dequantize run banded across 4 threads.
Gating softmax stays in fp32 (fp16 exp would overflow for |logit| > 11);
the u accumulator stays fp32; matmuls take fp16 operands with fp32 PSUM
accumulation.  If the fast runner hits any problem, kernel() falls back
to a plain run_bass_kernel_spmd call on the same NEFF.
"""

import numpy as np

B, D, R, E, L = 16384, 1024, 64, 4, 3
NCORES = 8
BC = B // NCORES            # batch rows per core
BT = 512                    # batch columns per chunk (fp32 PSUM bank capacity)
KC = D // 128               # K-chunks over D
NM = D // 128               # M-chunks over D

NSLICE = 1                  # single-shot: see transfer notes in docstring
BCS = BC // NSLICE          # rows per core per slice
ROWS = NCORES * BCS         # global rows per slice

QCLIP = 126.0               # int8 headroom so approx-reciprocal can't wrap

# packed fp16 weight blob [128, WCOLS] column offsets
OFF_GT = 0                  # [128, KC*E]        gating weights
OFF_BIA = OFF_GT + KC * E   # [128, L*NM]        bias (+1 folded into layer 0)
OFF_VP = OFF_BIA + L * NM   # L x [128, KC*2*128]
OFF_CB = OFF_VP + L * KC * 2 * 128   # L x [128, 2*128]
OFF_UP = OFF_CB + L * 2 * 128        # L x [128, 2*NM*128]
WCOLS = OFF_UP + L * 2 * NM * 128
WROWS_SH = 128 // NCORES    # blob rows uploaded per core

_CACHE = {}


def _build(bcs):
    import concourse.mybir as mybir
    import concourse.bacc as bacc
    import concourse.tile as tile

    bt = min(BT, bcs)       # batch columns per chunk
    nchunk = bcs // bt
    nbi = bt // 128         # 128-row blocks per chunk
    f32 = mybir.dt.float32
    f16 = mybir.dt.float16
    i8 = mybir.dt.int8
    ALU = mybir.AluOpType
    ACTF = mybir.ActivationFunctionType

    nc = bacc.Bacc("TRN2", target_bir_lowering=False, debug=False,
                   num_devices=NCORES)

    XQ = nc.dram_tensor("XQ", [bcs, D], i8, kind="ExternalInput")
    SX = nc.dram_tensor("SX", [bcs, 1], f16, kind="ExternalInput")
    WSH = nc.dram_tensor("WSH", [WROWS_SH, WCOLS], f16, kind="ExternalInput")
    UQ = nc.dram_tensor("UQ", [bcs, D], i8, kind="ExternalOutput")
    US = nc.dram_tensor("US", [bcs, 1], f16, kind="ExternalOutput")

    wbnc = nc.dram_tensor("wbnc", [WROWS_SH, WCOLS], f16)
    wall = nc.dram_tensor("wall", [128, WCOLS], f16, addr_space="Shared")

    # inline constants (shipped inside the NEFF, no upload)
    ident_h = np.eye(128, dtype=np.float16)
    sel_h = np.zeros((E, 2, 128), np.float32)
    for e in range(E):
        sel_h.reshape(E, 256)[e, e * 64:(e + 1) * 64] = 1.0
    ones_h = np.ones((E, E), np.float32)

    with tile.TileContext(nc) as tc:
        with (
            tc.tile_pool(name="wts", bufs=1) as wts,
            tc.tile_pool(name="xqp", bufs=2) as xqp,
            tc.tile_pool(name="xrp", bufs=2) as xrp,
            tc.tile_pool(name="sxp", bufs=2) as sxp,
            tc.tile_pool(name="y0p", bufs=3) as y0p,
            tc.tile_pool(name="yp", bufs=3) as yp,
            tc.tile_pool(name="up", bufs=2) as upool,
            tc.tile_pool(name="tp", bufs=2) as tp,
            tc.tile_pool(name="twp", bufs=2) as twp,
            tc.tile_pool(name="wgp", bufs=2) as wgp,
            tc.tile_pool(name="gp", bufs=2) as gp,
            tc.tile_pool(name="orp", bufs=2) as orp,
            tc.tile_pool(name="qsp", bufs=2) as qsp,
            tc.tile_pool(name="ps_g", bufs=1, space="PSUM") as ps_g,
            tc.tile_pool(name="ps_gbc", bufs=2, space="PSUM") as ps_gbc,
            tc.tile_pool(name="ps_vw", bufs=2, space="PSUM") as ps_vw,
            tc.tile_pool(name="ps_acc", bufs=2, space="PSUM") as ps_acc,
            tc.tile_pool(name="ps_tr", bufs=1, space="PSUM") as ps_tr,
        ):
            # gather the 8 weight shards over NeuronLink ASAP
            nc.sync.dma_start(out=wbnc[:, :], in_=WSH[:, :])
            nc.gpsimd.collective_compute(
                "AllGather", mybir.AluOpType.bypass,
                replica_groups=[list(range(NCORES))],
                ins=[wbnc.ap()], outs=[wall.ap()])

            id_sb = wts.tile([128, 128], f16, tag="id")
            nc.sync.dma_start(out=id_sb[:], in_=nc.inline_tensor(
                ident_h, name="ident")[:, :])
            sel_sb = wts.tile([E, 2, 128], f32, tag="sel")
            nc.sync.dma_start(out=sel_sb[:], in_=nc.inline_tensor(
                sel_h, name="sel")[:, :, :])
            ones_sb = wts.tile([E, E], f32, tag="ones")
            nc.sync.dma_start(out=ones_sb[:], in_=nc.inline_tensor(
                ones_h, name="ones")[:, :])

            # weight tiles from the gathered blob
            gt_sb = wts.tile([128, KC * E], f16, tag="gt")
            nc.sync.dma_start(out=gt_sb[:],
                              in_=wall[:, OFF_GT:OFF_GT + KC * E])
            bia16 = wts.tile([128, L * NM], f16, tag="bia16")
            nc.sync.dma_start(out=bia16[:],
                              in_=wall[:, OFF_BIA:OFF_BIA + L * NM])
            bia_sb = wts.tile([128, L * NM], f32, tag="bia")
            nc.scalar.activation(bia_sb[:], bia16[:], ACTF.Copy)
            vp_sb, cb_sb, up_sb = [], [], []
            for i in range(L):
                vp = wts.tile([128, KC * 2 * 128], f16, tag=f"vp{i}")
                nc.sync.dma_start(
                    out=vp[:], in_=wall[:, OFF_VP + i * KC * 256:
                                        OFF_VP + (i + 1) * KC * 256])
                vp_sb.append(vp)
                cb = wts.tile([128, 2 * 128], f16, tag=f"cb{i}")
                nc.sync.dma_start(
                    out=cb[:], in_=wall[:, OFF_CB + i * 256:
                                        OFF_CB + (i + 1) * 256])
                cb_sb.append(cb)
                up = wts.tile([128, 2 * NM * 128], f16, tag=f"up{i}")
                nc.sync.dma_start(
                    out=up[:], in_=wall[:, OFF_UP + i * NM * 256:
                                        OFF_UP + (i + 1) * NM * 256])
                up_sb.append(up)

            def load_chunk(cidx):
                """DMA 512 int8 rows, dequantize, transpose to y0."""
                r0 = cidx * bt
                xr = []
                for bi in range(nbi):
                    rs = slice(r0 + bi * 128, r0 + (bi + 1) * 128)
                    xq = xqp.tile([128, D], i8, tag=f"xq{bi}")
                    nc.sync.dma_start(out=xq[:], in_=XQ[rs, :])
                    sx16 = sxp.tile([128, 1], f16, tag=f"sx{bi}")
                    nc.sync.dma_start(out=sx16[:], in_=SX[rs, :])
                    sxf = sxp.tile([128, 1], f32, tag=f"sxf{bi}")
                    nc.scalar.activation(sxf[:], sx16[:], ACTF.Copy)
                    t_ = xrp.tile([128, D], f16, tag=f"xr{bi}")
                    nc.scalar.activation(t_[:], xq[:], ACTF.Copy,
                                         scale=sxf[:, 0:1])
                    xr.append(t_)
                y0 = y0p.tile([128, KC, bt], f16, tag="y0",
                              name=f"y0_{cidx}")
                for dj in range(KC):
                    pt = ps_tr.tile([128, D], f16, tag="tr", name="pt")
                    for bi in range(nbi):
                        nc.tensor.transpose(
                            pt[:, bi * 128:(bi + 1) * 128],
                            xr[bi][:, dj * 128:(dj + 1) * 128], id_sb[:])
                    nc.scalar.activation(y0[:, dj, :], pt[:, :bt], ACTF.Copy)
                return {"y0": y0, "y_in": y0, "u": None}

            def emit_layer(st, i):
                y0, y_in = st["y0"], st["y_in"]
                if i == 0:
                    st["u"] = upool.tile([128, NM, bt], f32, tag="u",
                                         name="u")
                u = st["u"]
                # --- S1 V-stage ---
                v_ps = [ps_vw.tile([128, bt], f32, tag="vw",
                                   name=f"v{pr_}") for pr_ in range(2)]
                for pr in range(2):
                    for k in range(KC):
                        nc.tensor.matmul(
                            v_ps[pr][:],
                            vp_sb[i][:, (k * 2 + pr) * 128:
                                     (k * 2 + pr + 1) * 128],
                            y_in[:, k, :],
                            start=(k == 0), stop=(k == KC - 1))
                # --- S0 gating logits ---
                glog = ps_g.tile([E, bt], f32, tag="g")
                for k in range(KC):
                    nc.tensor.matmul(glog[:], gt_sb[:, k * E:(k + 1) * E],
                                     y_in[:, k, :],
                                     start=(k == 0), stop=(k == KC - 1))
                t_sb = [tp.tile([128, bt], f16, tag="t",
                                name=f"t{pr_}") for pr_ in range(2)]
                for pr in range(2):
                    nc.scalar.activation(t_sb[pr][:], v_ps[pr][:], ACTF.Tanh)
                eg = gp.tile([E, bt], f32, tag="eg")
                nc.scalar.activation(eg[:], glog[:], ACTF.Exp)
                z4 = ps_g.tile([E, bt], f32, tag="g")
                nc.tensor.matmul(z4[:], ones_sb[:], eg[:], start=True,
                                 stop=True)
                rz4 = gp.tile([E, bt], f32, tag="rz", bufs=1)
                nc.vector.reciprocal_approx_fast(out=rz4[:], in_=z4[:])
                gn4 = gp.tile([E, bt], f32, tag="gn")
                nc.vector.tensor_mul(out=gn4[:], in0=eg[:], in1=rz4[:])
                # --- S2 C-stage ---
                w_ps = [ps_vw.tile([128, bt], f32, tag="vw",
                                   name=f"w{pr_}") for pr_ in range(2)]
                for pr in range(2):
                    nc.tensor.matmul(w_ps[pr][:],
                                     cb_sb[i][:, pr * 128:(pr + 1) * 128],
                                     t_sb[pr][:], start=True, stop=True)
                gbc_ps = [ps_gbc.tile([128, bt], f32, tag="gbc",
                                      name=f"gbc{pr_}") for pr_ in range(2)]
                for pr in range(2):
                    nc.tensor.matmul(gbc_ps[pr][:], sel_sb[:, pr, :],
                                     gn4[:], start=True, stop=True)
                wg_sb = []
                for pr in range(2):
                    tw = twp.tile([128, bt], f32, tag="tw")
                    nc.scalar.activation(tw[:], w_ps[pr][:], ACTF.Tanh)
                    wg = wgp.tile([128, bt], f16, tag="wg")
                    nc.vector.tensor_mul(out=wg[:], in0=tw[:],
                                         in1=gbc_ps[pr][:])
                    wg_sb.append(wg)
                # --- S3 U-stage + u update + y materialization ---
                last = (i == L - 1)
                if not last:
                    y_out = yp.tile([128, KC, bt], f16, tag="y")
                for m in range(NM):
                    acc = ps_acc.tile([128, bt], f32, tag="acc")
                    nc.tensor.matmul(acc[:],
                                     up_sb[i][:, m * 256:m * 256 + 128],
                                     wg_sb[0][:], start=True, stop=False)
                    nc.tensor.matmul(acc[:],
                                     up_sb[i][:, m * 256 + 128:m * 256 + 256],
                                     wg_sb[1][:], start=False, stop=True)
                    bcol = bia_sb[:, i * NM + m: i * NM + m + 1]
                    if i == 0:
                        # u = acc + (1 + bias_0)  (the 1+ is folded into BIA)
                        nc.scalar.activation(u[:, m, :], acc[:],
                                             ACTF.Identity, bias=bcol)
                    else:
                        nc.vector.scalar_tensor_tensor(
                            out=u[:, m, :], in0=acc[:], scalar=bcol,
                            in1=u[:, m, :], op0=ALU.add, op1=ALU.add)
                    if not last:
                        nc.vector.tensor_mul(out=y_out[:, m, :],
                                             in0=y0[:, m, :], in1=u[:, m, :])
                if not last:
                    st["y_in"] = y_out

            def store_chunk(st, cidx):
                """Quantize u int8 per batch row and DMA out with scales."""
                r0 = cidx * bt
                u = st["u"]
                uh = yp.tile([128, KC, bt], f16, tag="y", name="uh")
                for m in range(NM):
                    nc.scalar.activation(uh[:, m, :], u[:, m, :], ACTF.Copy)
                for bi in range(nbi):
                    rs = slice(r0 + bi * 128, r0 + (bi + 1) * 128)
                    po = ps_tr.tile([128, D], f16, tag="tr", name="po")
                    for dj in range(KC):
                        nc.tensor.transpose(
                            po[:, dj * 128:(dj + 1) * 128],
                            uh[:, dj, bi * 128:(bi + 1) * 128], id_sb[:])
                    m1 = qsp.tile([128, 1], f32, tag="m1")
                    nc.vector.tensor_reduce(
                        out=m1[:], in_=po[:, :],
                        axis=mybir.AxisListType.X,
                        op=ALU.max, apply_absolute_value=True)
                    nc.vector.tensor_scalar_max(out=m1[:], in0=m1[:],
                                                scalar1=1e-6)
                    rq = qsp.tile([128, 1], f32, tag="rq")
                    nc.vector.reciprocal(out=rq[:], in_=m1[:])
                    nc.vector.tensor_scalar_mul(out=rq[:], in0=rq[:],
                                                scalar1=QCLIP)
                    qo = orp.tile([128, D], i8, tag="or")
                    nc.scalar.activation(qo[:], po[:], ACTF.Copy,
                                         scale=rq[:, 0:1])
                    us16 = qsp.tile([128, 1], f16, tag="us")
                    nc.scalar.activation(us16[:], m1[:], ACTF.Copy,
                                         scale=1.0 / QCLIP)
                    nc.sync.dma_start(out=UQ[rs, :], in_=qo[:])
                    nc.sync.dma_start(out=US[rs, :], in_=us16[:])

            for cidx in range(nchunk):
                st = load_chunk(cidx)
                for i in range(L):
                    emit_layer(st, i)
                store_chunk(st, cidx)
    nc.compile()
    return nc


def _get_nc(bcs):
    key = ("nc", bcs)
    if key not in _CACHE:
        _CACHE[key] = _build(bcs)
    return _CACHE[key]


def _pack_weights(U, V, C, G, bias):
    """One fp16 blob [128, WCOLS] holding every weight in its SBUF layout."""
    W2 = np.zeros((128, WCOLS), np.float16)
    # gating [128, KC*E]: G.T [D, E] -> [KC, 128, E] -> [128, KC, E]
    W2[:, OFF_GT:OFF_GT + KC * E] = (
        G.T.reshape(KC, 128, E).transpose(1, 0, 2).reshape(128, KC * E))
    # bias [128, L*NM] with the residual "1 +" folded into layer 0
    biasm = bias.astype(np.float32, copy=True)
    biasm[0] += 1.0
    W2[:, OFF_BIA:OFF_BIA + L * NM] = (
        biasm.reshape(L, NM, 128).transpose(2, 0, 1).reshape(128, L * NM))
    # V packed pairs: [L, KC, 128, 2, 128] -> per layer [128, KC*2*128]
    VPh = V.transpose(0, 2, 1, 3).reshape(L, D, E * R).reshape(
        L, KC, 128, 2, 128)
    for i in range(L):
        W2[:, OFF_VP + i * KC * 256:OFF_VP + (i + 1) * KC * 256] = (
            VPh[i].transpose(1, 0, 2, 3).reshape(128, KC * 256))
    # C block-diagonal transposed: [L, 2, 128, 128] -> [128, 2*128]
    CBh = np.zeros((L, 2, 128, 128), np.float32)
    for i in range(L):
        for pr in range(2):
            CBh[i, pr, :64, :64] = C[i, 2 * pr].T
            CBh[i, pr, 64:, 64:] = C[i, 2 * pr + 1].T
    for i in range(L):
        W2[:, OFF_CB + i * 256:OFF_CB + (i + 1) * 256] = (
            CBh[i].transpose(1, 0, 2).reshape(128, 256))
    # U packed: [L, 2, 128, NM, 128] -> per layer [128, NM, 2, 128] flat,
    # matching the kernel's [:, m*256 + pr*128 + col] indexing
    UPh = U.transpose(0, 1, 3, 2).reshape(L, E * R, D).reshape(
        L, 2, 128, NM, 128)
    for i in range(L):
        W2[:, OFF_UP + i * NM * 256:OFF_UP + (i + 1) * NM * 256] = (
            UPh[i].transpose(1, 2, 0, 3).reshape(128, NM * 256))
    return W2


_HPOOL = None


def _hpool():
    """Thread pool for banded numpy work (ufuncs release the GIL)."""
    global _HPOOL
    if _HPOOL is None:
        import concurrent.futures as cf
        _HPOOL = cf.ThreadPoolExecutor(max_workers=4)
    return _HPOOL


def _banded(n, fn, bands=4):
    step = -(-n // bands)
    step = -(-step // 128) * 128
    futs = [_hpool().submit(fn, b0, min(b0 + step, n))
            for b0 in range(0, n, step)]
    for f in futs:
        f.result()


def _quantize_x(x):
    """Per-row symmetric int8: returns (int8 [n,D], fp16 scales [n,1])."""
    n = x.shape[0]
    q = np.empty(x.shape, np.int8)
    s = np.empty((n, 1), np.float16)

    def band(b0, b1):
        xb = x[b0:b1]
        sx = np.abs(xb).max(axis=1)
        np.maximum(sx, 1e-20, out=sx)
        t = xb * (QCLIP / sx)[:, None]
        np.rint(t, out=t)
        q[b0:b1] = t
        s[b0:b1, 0] = sx * (1.0 / QCLIP)

    _banded(n, band)
    return q, s


def _decode_u(out, r0, uq, us, x0):
    """out[r0:r0+n] = x0[r0:r0+n] * (uq * us), banded across threads."""
    n = uq.shape[0]
    usf = us.astype(np.float32)

    def band(b0, b1):
        np.multiply(uq[b0:b1], usf[b0:b1], out=out[r0 + b0:r0 + b1])
        np.multiply(out[r0 + b0:r0 + b1], x0[r0 + b0:r0 + b1],
                    out=out[r0 + b0:r0 + b1])

    _banded(n, band)


class _FastRunner:
    """Low-overhead executor built on the bass_exec PJRT primitive.

    Mirrors run_bass_via_pjrt's binding protocol exactly, but keeps the
    jitted executable cached across calls, keeps the weight blob
    device-resident (re-uploaded only when the weights change), and
    creates the donated output buffers on device instead of uploading
    host zeros.
    """

    def __init__(self):
        import concurrent.futures as cf
        import jax
        import concourse.mybir as mybir
        from jax.experimental.shard_map import shard_map
        from jax.sharding import Mesh, PartitionSpec, NamedSharding
        from concourse import bass2jax

        nc = _get_nc(BCS)
        if nc.dbg_addr is not None or nc.dbg_callbacks:
            raise RuntimeError("fast runner needs debug=False")
        bass2jax.install_neuronx_cc_hook()
        self._jax = jax

        partition_name = (nc.partition_id_tensor.name
                          if nc.partition_id_tensor else None)
        in_names, out_names, out_avals = [], [], []
        for alloc in nc.m.functions[0].allocations:
            if not isinstance(alloc, mybir.MemoryLocationSet):
                continue
            name = alloc.memorylocations[0].name
            if alloc.kind == "ExternalInput":
                if name != partition_name:
                    in_names.append(name)
            elif alloc.kind == "ExternalOutput":
                shape = tuple(alloc.tensor_shape)
                dtype = mybir.dt.np(alloc.dtype)
                out_names.append(name)
                out_avals.append(jax.core.ShapedArray(shape, dtype))
        assert set(in_names) == {"XQ", "SX", "WSH"}, in_names
        assert set(out_names) == {"UQ", "US"}, out_names
        n_params = len(in_names)
        self._arg_names = list(in_names)
        self._out_names = list(out_names)
        all_names = in_names + out_names
        if partition_name is not None:
            all_names = all_names + [partition_name]

        def _body(*args):
            operands = list(args)
            if partition_name is not None:
                operands.append(bass2jax.partition_id_tensor())
            outs = bass2jax._bass_exec_p.bind(
                *operands,
                out_avals=tuple(out_avals),
                in_names=tuple(all_names),
                out_names=tuple(out_names),
                lowering_input_output_aliases=(),
                sim_require_finite=True,
                sim_require_nnan=True,
                nc=nc,
            )
            return tuple(outs)

        devices = jax.devices()[:NCORES]
        assert len(devices) == NCORES, devices
        mesh = Mesh(np.asarray(devices), ("core",))
        self._sharding = NamedSharding(mesh, PartitionSpec("core"))
        n_all = n_params + len(out_names)
        in_specs = (PartitionSpec("core"),) * n_all
        out_specs = (PartitionSpec("core"),) * len(out_names)
        donate = tuple(range(n_params, n_all))
        self._jitted = jax.jit(
            shard_map(_body, mesh=mesh, in_specs=in_specs,
                      out_specs=out_specs, check_rep=False),
            donate_argnums=donate, keep_unused=True)
        zshapes = [((NCORES * a.shape[0],) + a.shape[1:], a.dtype)
                   for a in out_avals]
        self._zjit = jax.jit(
            lambda: tuple(jax.numpy.zeros(s, d) for s, d in zshapes),
            out_shardings=tuple(self._sharding for _ in zshapes))
        self._pool = cf.ThreadPoolExecutor(max_workers=2)
        self._wkey = None
        self._wdev = None
        # donation buffers for the next call: recycled from the previous
        # call's outputs (their host copies are fetched before run()
        # returns, so the device buffers are free to donate).  Creating
        # fresh zeros via _zjit costs a ~95ms RPC round trip per call.
        self._donor = None

    def get_wdev(self, U, V, C, G, bias):
        """Device-resident weight blob, reused across calls when unchanged."""
        parts = (U, V, C, G, bias)
        if self._wkey is not None and all(
                np.array_equal(a, b) for a, b in zip(self._wkey, parts)):
            return self._wdev
        W2 = _pack_weights(U, V, C, G, bias)
        self._wdev = self._jax.device_put(W2, self._sharding)
        self._wkey = tuple(p.copy() for p in parts)
        return self._wdev

    def run(self, wdev, x0):
        """Returns the final f32 output [B, D] = x0 * dequant(u)."""
        jax = self._jax
        futs = []
        last_outs = None
        for k in range(NSLICE):
            rows = slice(k * ROWS, (k + 1) * ROWS)
            xq_k, sx_k = _quantize_x(x0[rows])
            ops = {"XQ": jax.device_put(xq_k, self._sharding),
                   "SX": jax.device_put(sx_k, self._sharding),
                   "WSH": wdev}
            args = tuple(ops[n] for n in self._arg_names)
            donor, self._donor = self._donor, None
            if donor is None:
                donor = self._zjit()
            outs = self._jitted(*args, *donor)
            last_outs = outs
            byname = dict(zip(self._out_names, outs))
            futs.append((self._pool.submit(np.asarray, byname["UQ"]),
                         self._pool.submit(np.asarray, byname["US"])))
        out = np.empty((B, D), np.float32)
        for k, (fq, fs) in enumerate(futs):
            uq, us = fq.result(), fs.result()
            _decode_u(out, k * ROWS, uq, us, x0)
        self._donor = last_outs
        return out


def _get_fast():
    if "fast" not in _CACHE:
        _CACHE["fast"] = _FastRunner()
    return _CACHE["fast"]


def _run_fallback(xq, sx, W2, x0):
    """Plain single-shot path via run_bass_kernel_spmd (full batch)."""
    from concourse.bass_utils import run_bass_kernel_spmd
    nc = _get_nc(BC)
    in_maps = []
    for c in range(NCORES):
        rows = slice(c * BC, (c + 1) * BC)
        in_maps.append({
            "XQ": xq[rows], "SX": sx[rows],
            "WSH": W2[c * WROWS_SH:(c + 1) * WROWS_SH],
        })
    res = run_bass_kernel_spmd(nc, in_maps, core_ids=list(range(NCORES)))
    out = np.empty((B, D), np.float32)
    for c in range(NCORES):
        rows = slice(c * BC, (c + 1) * BC)
        uq = res.results[c]["UQ"]
        us = res.results[c]["US"]
        np.multiply(uq, us.astype(np.float32), out=out[rows])
        np.multiply(out[rows], x0[rows], out=out[rows])
    return out


def kernel(inputs, U, V, C, G, bias):
    inputs = np.asarray(inputs, dtype=np.float32)
    U = np.asarray(U, dtype=np.float32)
    V = np.asarray(V, dtype=np.float32)
    C = np.asarray(C, dtype=np.float32)
    G = np.asarray(G, dtype=np.float32)
    bias = np.asarray(bias, dtype=np.float32)

    try:
        fast = _get_fast()
        return fast.run(fast.get_wdev(U, V, C, G, bias), inputs)
    except Exception:
        import sys, traceback
        traceback.print_exc()
        print("kernel: fast path failed; using fallback", file=sys.stderr)
        xq, sx = _quantize_x(inputs)
        W2 = _pack_weights(U, V, C, G, bias)
        return _run_fallback(xq, sx, W2, inputs)

